# revision 5
# baseline (speedup 1.0000x reference)
# Trainium2 Bass kernel for nn_LocalCrossAttentionModule.
#
# Math: softmax over a size-1 axis is identically 1, so q/k (and x_query,
# Wq, bq, Wk, bk) never affect the output. The module reduces to, per
# 5x5 patch p (576 of them = 4 batch x 12x12 grid, stride 36):
#   kvf_p  = flatten(x_kv patch)                  (3200,)
#   v_p    = Wv @ kvf_p + bv                      (1600,) viewed as (64, 5, 5)
#   z_p    = conv_w @ v_p[:, s] + conv_b          (128,) per pixel s in 5x5
# z_p is scattered into an otherwise-constant (conv_b) output map.
#
# Sharding: 4 feature-shards x 2 patch-halves across 8 cores. Each
# feature-shard owns 6 whole patch pixels (24 of 25); the 25th pixel is
# handled by folding the 1x1 conv into the weights host-side
# (W2 = conv_w @ Wv_p24), splitting its 128 conv-output rows 32 per
# feature-shard. Per-core device work is one fused [128, 25, 704] f16
# stream (weights 416 cols | kvf half 288 cols per k-tile).
#
# The device program is raw bacc (no TileContext): Tile's end-of-kernel
# semaphore butterfly costs ~10us of HW exec time, so semaphores are
# placed by hand (7 sems).

import numpy as np

B = 4
CKV = 128
HW_ = 432
E = 2
PP = 5           # patch side
STRIDE = 36
PI = 12          # patch grid side
NP = B * PI * PI      # 576 patches
KF = CKV * PP * PP    # 3200 kv features per patch
KT = KF // 128        # 25 contraction k-tiles
OUT = 64
O2 = 128
NCORES = 8
NF = 4           # feature shards
NPX = 6          # whole pixels per feature shard
WCOLS = NPX * OUT + 32   # 416 weight cols (384 v-rows + 32 folded z-rows)
NCH = NP // 2    # 288 patches per core (half)
WKC = WCOLS + NCH        # 704 (per k-tile: [kvf 288 | w 416])
CHUNKS = [1, 2, 3, 4, 4, 4, 3, 2, 1, 1]  # k-tiles per input DMA (sum 25)
N_WARM = 14      # PE warm-up matmuls (cold ~240ns each ~ 3.4us window)

_PROGRAM = {}


def _build_program():
    import concourse.mybir as mybir
    from concourse import bacc

    f32 = mybir.dt.float32
    f16 = mybir.dt.float16
    add = mybir.AluOpType.add

    nc = bacc.Bacc()
    wk_d = nc.declare_dram_parameter("wk", [128, KT, WKC], f16, isOutput=False)
    cb_d = nc.declare_dram_parameter("cb", [128, 5], f32, isOutput=False)
    cw_d = nc.declare_dram_parameter("cw", [128, 128], f16, isOutput=False)
    z_d = nc.declare_dram_parameter("z", [128, NPX, NCH], f16, isOutput=True)
    z24_d = nc.declare_dram_parameter("z24", [32, NCH], f16, isOutput=True)

    # chunk index that must be complete before k-tile k is consumed
    need = []
    for ci, sz in enumerate(CHUNKS):
        need += [ci] * sz

    from contextlib import ExitStack

    with ExitStack() as stack:
        ec = stack.enter_context
        s_in = ec(nc.semaphore("s_in"))      # wk chunk completions (x16)
        s_c = ec(nc.semaphore("s_c"))        # const DMA completions (x16)
        s_warm = ec(nc.semaphore("s_warm"))  # warm tile memset done
        s_pe = ec(nc.semaphore("s_pe"))      # mm1 accumulation done per m
        s_v = ec(nc.semaphore("s_v"))        # V extracted per m
        s_pe2 = ec(nc.semaphore("s_pe2"))    # mm2 done per pixel
        s_z = ec(nc.semaphore("s_z"))        # z written per pixel
        s_z24 = ec(nc.semaphore("s_z24"))    # z24 written
        s_out = ec(nc.semaphore("s_out"))    # output DMA completions (x16)
        # flat layout; +128 pad cols so the last k-tile's widened m3
        # lhsT read (spills past the weights) stays in-bounds
        wk_t = ec(nc.sbuf_tensor("wk_t", [128, KT * WKC + 128], f16))
        cb_t = ec(nc.sbuf_tensor("cb_t", [128, 5], f32))
        cw_t = ec(nc.sbuf_tensor("cw_t", [128, 128], f16))
        warm_t = ec(nc.sbuf_tensor("warm_t", [128, NCH], f16))
        v_t = ec(nc.sbuf_tensor("v_t", [128, 3, NCH], f16))
        z_t = ec(nc.sbuf_tensor("z_t", [128, NPX, NCH], f16))
        z24_t = ec(nc.sbuf_tensor("z24_t", [32, NCH], f16))
        psv0 = ec(nc.psum_tensor("psv0", [128, NCH], f32))
        psv1 = ec(nc.psum_tensor("psv1", [128, NCH], f32))
        psv2 = ec(nc.psum_tensor("psv2", [128, NCH], f32))
        psv3 = ec(nc.psum_tensor("psv3", [128, NCH], f32))
        psz0 = ec(nc.psum_tensor("psz0", [128, NCH], f32))
        psz1 = ec(nc.psum_tensor("psz1", [128, NCH], f32))
        psz2 = ec(nc.psum_tensor("psz2", [128, NCH], f32))
        psz3 = ec(nc.psum_tensor("psz3", [128, NCH], f32))
        ps_v = [psv0, psv1, psv2, psv3]
        # mm2 output banks: 4 fresh + reuse psv0/psv1 (their V is long
        # extracted by the time pixels 4/5 run, guarded by s_v)
        ps_z = [psz0, psz1, psz2, psz3, psv0, psv1]
        all_sems = [s_in, s_c, s_warm, s_pe, s_v, s_pe2, s_z, s_z24, s_out]

        with nc.Block() as block:

            @block.sync
            def _(sync):
                lo = 0
                for sz in CHUNKS:
                    sync.dma_start(
                        wk_t[:, lo * WKC:(lo + sz) * WKC],
                        wk_d[:, lo:lo + sz, :],
                    ).then_inc(s_in, 16)
                    lo += sz

            @block.scalar
            def _(scalar):
                scalar.dma_start(cb_t[:], cb_d[:]).then_inc(s_c, 16)
                scalar.dma_start(cw_t[:], cw_d[:]).then_inc(s_c, 16)
                # stores, earliest-ready first
                scalar.wait_ge(s_z24, 1)
                scalar.dma_start(z24_d[:], z24_t[:]).then_inc(s_out, 16)
                scalar.wait_ge(s_z, 2)
                scalar.dma_start(z_d[:, 0:2, :], z_t[:, 0:2, :]).then_inc(s_out, 16)
                scalar.wait_ge(s_z, 4)
                scalar.dma_start(z_d[:, 2:4, :], z_t[:, 2:4, :]).then_inc(s_out, 16)
                scalar.wait_ge(s_z, 6)
                scalar.dma_start(z_d[:, 4:6, :], z_t[:, 4:6, :]).then_inc(s_out, 16)

            @block.tensor
            def _(tensor):
                # warm-up: keeps PE_HAM busy through the first-chunk DMA
                # latency so real matmuls run at 2.4 GHz
                tensor.wait_ge(s_warm, 1)
                for _ in range(N_WARM):
                    tensor.matmul(
                        psz0[:], lhsT=warm_t[:, 0:128], rhs=warm_t[:],
                        start=True, stop=True,
                    )
                # mm1: V[f, n] accumulated over 25 k-tiles
                last_need = -1
                for k in range(KT):
                    if need[k] != last_need:
                        tensor.wait_ge(s_in, 16 * (need[k] + 1))
                        last_need = need[k]
                    base = k * WKC
                    for m in range(4):
                        # m3 is only 32 real cols (the conv-folded rows);
                        # widen to 128 by reading into the next k-tile's kvf
                        # region -- finite junk that lands in PSUM
                        # partitions 32-127, which nothing reads.
                        lo = base + NCH + m * 128
                        mm = tensor.matmul(
                            ps_v[m][:],
                            lhsT=wk_t[:, lo:lo + 128],
                            rhs=wk_t[:, base:base + NCH],
                            start=(k == 0),
                            stop=(k == KT - 1),
                        )
                        if k == KT - 1:
                            mm.then_inc(s_pe, 1)
                # mm2: z[o2, n] per pixel, contraction over 64 v-features
                tensor.wait_ge(s_c, 32)
                for p in range(NPX):
                    m, h = divmod(p, 2)
                    if h == 0:
                        tensor.wait_ge(s_v, m + 1)
                    tensor.matmul(
                        ps_z[p][:],
                        lhsT=cw_t[64 * h:64 * (h + 1), :],
                        rhs=v_t[64 * h:64 * (h + 1), m, :],
                        start=True, stop=True,
                    ).then_inc(s_pe2, 1)

            @block.vector
            def _(vector):
                vector.memset(wk_t[:, KT * WKC:], 0.0)
                vector.memset(warm_t[:], 0.0).then_inc(s_warm, 1)
                vector.wait_ge(s_c, 16)
                for m in range(3):
                    vector.wait_ge(s_pe, m + 1)
                    vector.tensor_tensor(
                        out=v_t[:, m, :],
                        in0=ps_v[m][:],
                        in1=cb_t[:, m:m + 1].to_broadcast((128, NCH)),
                        op=add,
                    ).then_inc(s_v, 1)
                vector.wait_ge(s_pe, 4)
                vector.tensor_tensor(
                    out=z24_t[:],
                    in0=ps_v[3][0:32, :],
                    in1=cb_t[0:32, 3:4].to_broadcast((32, NCH)),
                    op=add,
                ).then_inc(s_z24, 1)
                for p in range(NPX):
                    vector.wait_ge(s_pe2, p + 1)
                    vector.tensor_tensor(
                        out=z_t[:, p, :],
                        in0=ps_z[p][:],
                        in1=cb_t[:, 4:5].to_broadcast((128, NCH)),
                        op=add,
                    ).then_inc(s_z, 1)

            @block.gpsimd
            def _(gpsimd):
                # end-of-kernel: wait for all stores, then restore sem state
                gpsimd.wait_ge(s_out, 16 * 4)
                nums = sorted(s.num for s in all_sems)
                lo, hi = nums[0], nums[-1]
                assert nums == list(range(lo, hi + 1))
                gpsimd.dma_reset(range(lo, hi + 1))
                gpsimd.sem_clear(range(lo, hi + 1))

    nc.finalize()
    return nc


def _get_program():
    if "p" not in _PROGRAM:
        _PROGRAM["p"] = _build_program()
    return _PROGRAM["p"]


def _prep_in_maps(x_kv, Wv, bv, conv_w, conv_b):
    """Host-side shard/layout prep. Returns list of per-core input dicts."""
    x_kv = np.ascontiguousarray(np.asarray(x_kv, dtype=np.float32))
    Wv = np.asarray(Wv, dtype=np.float32)
    bv = np.asarray(bv, dtype=np.float32)
    conv_w = np.asarray(conv_w, dtype=np.float32)
    conv_b = np.asarray(conv_b, dtype=np.float32)

    # gather all 5x5 patches (padded coords: top-left of patch (pi,pj) is
    # original coords (pi*36-2, pj*36-2))
    pad = np.zeros((B, CKV, HW_ + 2 * E, HW_ + 2 * E), np.float32)
    pad[:, :, E:HW_ + E, E:HW_ + E] = x_kv
    r = (np.arange(PI)[:, None] * STRIDE + np.arange(PP)).ravel()  # (60,)
    g = pad[:, :, r[:, None], r[None, :]]                # (B, C, 60, 60)
    g = g.reshape(B, CKV, PI, PP, PI, PP)
    # feature j = c*25 + pr*5 + pc ; patch n = b*144 + pi*12 + pj
    kvf_t = g.transpose(1, 3, 5, 0, 2, 4).reshape(KF, NP)     # (3200, 576)
    kv_arr = kvf_t.reshape(KT, 128, NP).transpose(1, 0, 2)    # (128, 25, 576)
    kv_arr = np.ascontiguousarray(kv_arr).astype(np.float16)

    # conv folded into the 25th pixel's weights
    perm24 = np.array([o * PP * PP + 24 for o in range(OUT)], np.int64)
    W2 = conv_w @ Wv[perm24]                 # (128, 3200)
    b2 = conv_w @ bv[perm24] + conv_b        # (128,)

    # conv_w.T duplicated into both partition halves (mm2 lhsT must share
    # the rhs base partition)
    cw = np.ascontiguousarray(
        np.concatenate([conv_w.T, conv_w.T], axis=0)).astype(np.float16)

    in_maps = [None] * NCORES
    for f in range(NF):
        pixels = range(NPX * f, NPX * (f + 1))
        perm = np.array(
            [o * PP * PP + s for s in pixels for o in range(OUT)], np.int64
        )  # 384, layout j = s_local*64 + o
        A = np.concatenate([Wv[perm], W2[32 * f:32 * (f + 1)]], axis=0)  # (416, 3200)
        lhsT = np.ascontiguousarray(A.T)                     # (3200, 416)
        w_arr = lhsT.reshape(KT, 128, WCOLS).transpose(1, 0, 2)  # (128, 25, 416)
        w_arr = np.ascontiguousarray(w_arr).astype(np.float16)

        cb = np.zeros((128, 5), np.float32)
        cb[:, 0:3] = bv[perm].reshape(3, 128).T
        cb[0:32, 3] = b2[32 * f:32 * (f + 1)]
        cb[:, 4] = conv_b

        for p in range(2):
            wk = np.concatenate(
                [kv_arr[:, :, NCH * p:NCH * (p + 1)], w_arr], axis=2
            )  # (128, 25, 704) f16, [kvf | w] per k-tile
            in_maps[2 * f + p] = {
                "wk": np.ascontiguousarray(wk),
                "cb": cb,
                "cw": cw,
            }
    return in_maps


def _assemble(results, conv_b, out_dtype=np.float32):
    """Scatter per-core z outputs into the full (B, 128, 432, 432) map."""
    conv_b = np.asarray(conv_b, dtype=np.float32)
    y = np.empty((B, O2, HW_, HW_), np.float32)
    y[:] = conv_b.reshape(1, O2, 1, 1)
    base = np.arange(PI) * STRIDE
    for c in range(NCORES):
        f, p = divmod(c, 2)
        bs = slice(2 * p, 2 * p + 2)  # patch half p covers batches 2p, 2p+1
        z = np.asarray(results[c]["z"], np.float32)      # (128, 6, 288)
        for sl, s in enumerate(range(NPX * f, NPX * (f + 1))):
            pr, pc = divmod(s, PP)
            blk = z[:, sl, :].reshape(O2, 2, PI, PI).transpose(1, 0, 2, 3)
            y[bs, :, (base + pr)[:, None], (base + pc)[None, :]] = blk
        z24 = np.asarray(results[c]["z24"], np.float32)  # (32, 288)
        blk = z24.reshape(32, 2, PI, PI).transpose(1, 0, 2, 3)
        y[bs, 32 * f:32 * (f + 1),
          (base + PP - 1)[:, None], (base + PP - 1)[None, :]] = blk
    return y.astype(out_dtype, copy=False)


def _run(inputs, trace=False, trace_kwargs=None):
    from concourse.bass_utils import run_bass_kernel_spmd

    in_maps = _prep_in_maps(
        inputs["x_kv"], inputs["Wv"], inputs["bv"],
        inputs["conv_w"], inputs["conv_b"],
    )
    nc = _get_program()
    kw = {}
    if trace:
        kw["trace"] = True
        if trace_kwargs:
            kw.update(trace_kwargs)
    res = run_bass_kernel_spmd(nc, in_maps, list(range(NCORES)), **kw)
    out = _assemble(res.results, inputs["conv_b"])
    return out, res


def kernel(**inputs):
    out, _ = _run(inputs, trace=False)
    return out


# revision 7
# speedup vs baseline: 1.1458x; 1.1458x over previous
# Trainium2 Bass kernel for nn_LocalCrossAttentionModule.
#
# Math: softmax over a size-1 axis is identically 1, so q/k (and x_query,
# Wq, bq, Wk, bk) never affect the output. The module reduces to, per
# 5x5 patch p (576 of them = 4 batch x 12x12 grid, stride 36):
#   kvf_p  = flatten(x_kv patch)                  (3200,)
#   v_p    = Wv @ kvf_p + bv                      (1600,) viewed as (64, 5, 5)
#   z_p    = conv_w @ v_p[:, s] + conv_b          (128,) per pixel s in 5x5
# z_p is scattered into an otherwise-constant (conv_b) output map.
#
# Sharding: 4 feature-shards x 2 patch-halves across 8 cores. Each
# feature-shard owns 6 whole patch pixels (24 of 25); the 25th pixel is
# handled by folding the 1x1 conv into the weights host-side
# (W2 = conv_w @ Wv_p24), splitting its 128 conv-output rows 32 per
# feature-shard. Per-core device work is one fused [128, 25, 704] f16
# stream (weights 416 cols | kvf half 288 cols per k-tile).
#
# The device program is raw bacc (no TileContext): Tile's end-of-kernel
# semaphore butterfly costs ~10us of HW exec time, so semaphores are
# placed by hand (7 sems).

import numpy as np

B = 4
CKV = 128
HW_ = 432
E = 2
PP = 5           # patch side
STRIDE = 36
PI = 12          # patch grid side
NP = B * PI * PI      # 576 patches
KF = CKV * PP * PP    # 3200 kv features per patch
KT = KF // 128        # 25 contraction k-tiles
OUT = 64
O2 = 128
NCORES = 8
NF = 4           # feature shards
NPX = 6          # whole pixels per feature shard
WCOLS = NPX * OUT + 32   # 416 weight cols (384 v-rows + 32 folded z-rows)
NCH = NP // 2    # 288 patches per core (half)
WKC = WCOLS + NCH        # 704 (per k-tile: [kvf 288 | w 416])
CHUNKS = [1, 2, 4, 5, 5, 4, 2, 1, 1]  # k-tiles per input DMA (sum 25)
N_WARM = 13      # PE warm-up matmuls (cold ~240ns each ~ 3.1us)

_PROGRAM = {}


def _build_program():
    import concourse.mybir as mybir
    from concourse import bacc
    from concourse import bass as bassmod
    from contextlib import contextmanager

    @contextmanager
    def open_block(nc, name):
        # BassBlock without the exit all_engine_barrier: each engine flows
        # straight into the framework's end-of-kernel semaphore walk when
        # its own stream ends, overlapping the walk with other engines'
        # tails. Engines whose walk subset contains our sems (GpSimd:
        # 105-155, Vector: 156-206) end with a wait on store completion.
        assert nc.cur_block is None
        blk = bassmod.BassBlock(nc, name)
        blk.__enter__()
        nc.cur_block = blk
        try:
            yield blk
        finally:
            for engine, last_body in blk.last_body.items():
                with nc.body(last_body, parent=nc.cur_bb,
                             allow_existing_parent=True):
                    engine.br(blk.end_bb)
            nc.switch_bb(blk.end_bb)
            nc.cur_block = None

    f32 = mybir.dt.float32
    f16 = mybir.dt.float16
    add = mybir.AluOpType.add

    nc = bacc.Bacc()
    wk_d = nc.declare_dram_parameter("wk", [128, KT, WKC], f16, isOutput=False)
    cb_d = nc.declare_dram_parameter("cb", [128, 5], f32, isOutput=False)
    cw_d = nc.declare_dram_parameter("cw", [128, 128], f16, isOutput=False)
    z_d = nc.declare_dram_parameter("z", [128, NPX, NCH], f16, isOutput=True)
    z24_d = nc.declare_dram_parameter("z24", [32, NCH], f16, isOutput=True)

    # chunk index that must be complete before k-tile k is consumed
    need = []
    for ci, sz in enumerate(CHUNKS):
        need += [ci] * sz

    from contextlib import ExitStack

    with ExitStack() as stack:
        ec = stack.enter_context
        s_in = ec(nc.semaphore("s_in"))      # wk chunk completions (x16)
        s_c = ec(nc.semaphore("s_c"))        # const DMA completions (x16)
        s_pad = ec(nc.semaphore("s_pad"))    # junk-region memsets done
        s_pe = ec(nc.semaphore("s_pe"))      # mm1 accumulation done per m
        s_v = ec(nc.semaphore("s_v"))        # V extracted per m
        s_pe2 = ec(nc.semaphore("s_pe2"))    # mm2 done per pixel
        s_z = ec(nc.semaphore("s_z"))        # z written per pixel
        s_z24 = ec(nc.semaphore("s_z24"))    # z24 written
        s_out = ec(nc.semaphore("s_out"))    # output DMA completions (x16)
        # flat layout; +128 pad cols so the last k-tile's widened m3
        # lhsT read (spills past the weights) stays in-bounds
        wk_t = ec(nc.sbuf_tensor("wk_t", [128, KT * WKC + 128], f16))
        cb_t = ec(nc.sbuf_tensor("cb_t", [128, 5], f32))
        cw_t = ec(nc.sbuf_tensor("cw_t", [128, 128], f16))
        warm_t = ec(nc.sbuf_tensor("warm_t", [128, NCH], f16))
        v_t = ec(nc.sbuf_tensor("v_t", [128, 3, NCH], f16))
        z_t = ec(nc.sbuf_tensor("z_t", [128, NPX, NCH], f16))
        z24_t = ec(nc.sbuf_tensor("z24_t", [32, NCH], f16))
        psv0 = ec(nc.psum_tensor("psv0", [128, NCH], f32))
        psv1 = ec(nc.psum_tensor("psv1", [128, NCH], f32))
        psv2 = ec(nc.psum_tensor("psv2", [128, NCH], f32))
        psv3 = ec(nc.psum_tensor("psv3", [128, NCH], f32))
        psz0 = ec(nc.psum_tensor("psz0", [128, NCH], f32))
        psz1 = ec(nc.psum_tensor("psz1", [128, NCH], f32))
        psz2 = ec(nc.psum_tensor("psz2", [128, NCH], f32))
        psz3 = ec(nc.psum_tensor("psz3", [128, NCH], f32))
        ps_v = [psv0, psv1, psv2, psv3]
        # mm2 output banks: 4 fresh + reuse psv0/psv1 (their V is long
        # extracted by the time pixels 4/5 run, guarded by s_v)
        ps_z = [psz0, psz1, psz2, psz3, psv0, psv1]

        with open_block(nc, "blk") as block:

            @block.sync
            def _(sync):
                lo = 0
                for sz in CHUNKS:
                    sync.dma_start(
                        wk_t[:, lo * WKC:(lo + sz) * WKC],
                        wk_d[:, lo:lo + sz, :],
                    ).then_inc(s_in, 16)
                    lo += sz

            @block.scalar
            def _(scalar):
                scalar.dma_start(cb_t[:], cb_d[:]).then_inc(s_c, 16)
                scalar.dma_start(cw_t[:], cw_d[:]).then_inc(s_c, 16)
                # stores, earliest-ready first
                scalar.wait_ge(s_z24, 1)
                scalar.dma_start(z24_d[:], z24_t[:]).then_inc(s_out, 16)
                scalar.wait_ge(s_z, 2)
                scalar.dma_start(z_d[:, 0:2, :], z_t[:, 0:2, :]).then_inc(s_out, 16)
                scalar.wait_ge(s_z, 4)
                scalar.dma_start(z_d[:, 2:4, :], z_t[:, 2:4, :]).then_inc(s_out, 16)
                scalar.wait_ge(s_z, 6)
                scalar.dma_start(z_d[:, 4:6, :], z_t[:, 4:6, :]).then_inc(s_out, 16)
                scalar.wait_ge(s_out, 16 * 4)

            @block.tensor
            def _(tensor):
                # warm-up: keeps PE_HAM busy through the first-chunk DMA
                # latency so real matmuls run at 2.4 GHz
                for _ in range(N_WARM):
                    tensor.matmul(
                        psz0[:], lhsT=warm_t[:, 0:128], rhs=warm_t[:],
                        start=True, stop=True,
                    )
                # mm1: V[f, n] accumulated over 25 k-tiles
                tensor.wait_ge(s_pad, 1)
                last_need = -1
                for k in range(KT):
                    if need[k] != last_need:
                        tensor.wait_ge(s_in, 16 * (need[k] + 1))
                        last_need = need[k]
                    base = k * WKC
                    for m in range(4):
                        # m3 is only 32 real cols (the conv-folded rows);
                        # widen to 128 by reading into the next k-tile's kvf
                        # region -- finite junk that lands in PSUM
                        # partitions 32-127, which nothing reads.
                        lo = base + NCH + m * 128
                        mm = tensor.matmul(
                            ps_v[m][:],
                            lhsT=wk_t[:, lo:lo + 128],
                            rhs=wk_t[:, base:base + NCH],
                            start=(k == 0),
                            stop=(k == KT - 1),
                        )
                        if k == KT - 1:
                            mm.then_inc(s_pe, 1)
                # mm2: z[o2, n] per pixel, contraction over 64 v-features
                tensor.wait_ge(s_c, 32)
                for p in range(NPX):
                    m, h = divmod(p, 2)
                    if h == 0:
                        tensor.wait_ge(s_v, m + 1)
                    tensor.matmul(
                        ps_z[p][:],
                        lhsT=cw_t[64 * h:64 * (h + 1), :],
                        rhs=v_t[64 * h:64 * (h + 1), m, :],
                        start=True, stop=True,
                    ).then_inc(s_pe2, 1)
                # the end-of-kernel walk clears runtime sems (2-53 on this
                # engine); hold it until all output DMA completed
                tensor.wait_ge(s_out, 16 * 4)

            @block.vector
            def _(vector):
                vector.memset(warm_t[:], 0.0)
                vector.memset(wk_t[:, KT * WKC:], 0.0).then_inc(s_pad, 1)
                vector.wait_ge(s_c, 16)
                for m in range(3):
                    vector.wait_ge(s_pe, m + 1)
                    vector.tensor_tensor(
                        out=v_t[:, m, :],
                        in0=ps_v[m][:],
                        in1=cb_t[:, m:m + 1].to_broadcast((128, NCH)),
                        op=add,
                    ).then_inc(s_v, 1)
                vector.wait_ge(s_pe, 4)
                vector.tensor_tensor(
                    out=z24_t[:],
                    in0=ps_v[3][0:32, :],
                    in1=cb_t[0:32, 3:4].to_broadcast((32, NCH)),
                    op=add,
                ).then_inc(s_z24, 1)
                for p in range(NPX):
                    vector.wait_ge(s_pe2, p + 1)
                    vector.tensor_tensor(
                        out=z_t[:, p, :],
                        in0=ps_z[p][:],
                        in1=cb_t[:, 4:5].to_broadcast((128, NCH)),
                        op=add,
                    ).then_inc(s_z, 1)
                vector.wait_ge(s_out, 16 * 4)

            @block.gpsimd
            def _(gpsimd):
                # hold GpSimd's end-walk (clears sems 105-155) until all
                # output DMAs completed; the walk itself restores sem state
                gpsimd.wait_ge(s_out, 16 * 4)

    nc.finalize()
    return nc


def _get_program():
    if "p" not in _PROGRAM:
        _PROGRAM["p"] = _build_program()
    return _PROGRAM["p"]


def _prep_in_maps(x_kv, Wv, bv, conv_w, conv_b):
    """Host-side shard/layout prep. Returns list of per-core input dicts."""
    x_kv = np.ascontiguousarray(np.asarray(x_kv, dtype=np.float32))
    Wv = np.asarray(Wv, dtype=np.float32)
    bv = np.asarray(bv, dtype=np.float32)
    conv_w = np.asarray(conv_w, dtype=np.float32)
    conv_b = np.asarray(conv_b, dtype=np.float32)

    # gather all 5x5 patches (padded coords: top-left of patch (pi,pj) is
    # original coords (pi*36-2, pj*36-2))
    pad = np.zeros((B, CKV, HW_ + 2 * E, HW_ + 2 * E), np.float32)
    pad[:, :, E:HW_ + E, E:HW_ + E] = x_kv
    r = (np.arange(PI)[:, None] * STRIDE + np.arange(PP)).ravel()  # (60,)
    g = pad[:, :, r[:, None], r[None, :]]                # (B, C, 60, 60)
    g = g.reshape(B, CKV, PI, PP, PI, PP)
    # feature j = c*25 + pr*5 + pc ; patch n = b*144 + pi*12 + pj
    kvf_t = g.transpose(1, 3, 5, 0, 2, 4).reshape(KF, NP)     # (3200, 576)
    kv_arr = kvf_t.reshape(KT, 128, NP).transpose(1, 0, 2)    # (128, 25, 576)
    kv_arr = np.ascontiguousarray(kv_arr).astype(np.float16)

    # conv folded into the 25th pixel's weights
    perm24 = np.array([o * PP * PP + 24 for o in range(OUT)], np.int64)
    W2 = conv_w @ Wv[perm24]                 # (128, 3200)
    b2 = conv_w @ bv[perm24] + conv_b        # (128,)

    # conv_w.T duplicated into both partition halves (mm2 lhsT must share
    # the rhs base partition)
    cw = np.ascontiguousarray(
        np.concatenate([conv_w.T, conv_w.T], axis=0)).astype(np.float16)

    in_maps = [None] * NCORES
    for f in range(NF):
        pixels = range(NPX * f, NPX * (f + 1))
        perm = np.array(
            [o * PP * PP + s for s in pixels for o in range(OUT)], np.int64
        )  # 384, layout j = s_local*64 + o
        A = np.concatenate([Wv[perm], W2[32 * f:32 * (f + 1)]], axis=0)  # (416, 3200)
        lhsT = np.ascontiguousarray(A.T)                     # (3200, 416)
        w_arr = lhsT.reshape(KT, 128, WCOLS).transpose(1, 0, 2)  # (128, 25, 416)
        w_arr = np.ascontiguousarray(w_arr).astype(np.float16)

        cb = np.zeros((128, 5), np.float32)
        cb[:, 0:3] = bv[perm].reshape(3, 128).T
        cb[0:32, 3] = b2[32 * f:32 * (f + 1)]
        cb[:, 4] = conv_b

        for p in range(2):
            wk = np.concatenate(
                [kv_arr[:, :, NCH * p:NCH * (p + 1)], w_arr], axis=2
            )  # (128, 25, 704) f16, [kvf | w] per k-tile
            in_maps[2 * f + p] = {
                "wk": np.ascontiguousarray(wk),
                "cb": cb,
                "cw": cw,
            }
    return in_maps


def _assemble(results, conv_b, out_dtype=np.float32):
    """Scatter per-core z outputs into the full (B, 128, 432, 432) map."""
    conv_b = np.asarray(conv_b, dtype=np.float32)
    y = np.empty((B, O2, HW_, HW_), np.float32)
    y[:] = conv_b.reshape(1, O2, 1, 1)
    base = np.arange(PI) * STRIDE
    for c in range(NCORES):
        f, p = divmod(c, 2)
        bs = slice(2 * p, 2 * p + 2)  # patch half p covers batches 2p, 2p+1
        z = np.asarray(results[c]["z"], np.float32)      # (128, 6, 288)
        for sl, s in enumerate(range(NPX * f, NPX * (f + 1))):
            pr, pc = divmod(s, PP)
            blk = z[:, sl, :].reshape(O2, 2, PI, PI).transpose(1, 0, 2, 3)
            y[bs, :, (base + pr)[:, None], (base + pc)[None, :]] = blk
        z24 = np.asarray(results[c]["z24"], np.float32)  # (32, 288)
        blk = z24.reshape(32, 2, PI, PI).transpose(1, 0, 2, 3)
        y[bs, 32 * f:32 * (f + 1),
          (base + PP - 1)[:, None], (base + PP - 1)[None, :]] = blk
    return y.astype(out_dtype, copy=False)


def _run(inputs, trace=False, trace_kwargs=None):
    from concourse.bass_utils import run_bass_kernel_spmd

    in_maps = _prep_in_maps(
        inputs["x_kv"], inputs["Wv"], inputs["bv"],
        inputs["conv_w"], inputs["conv_b"],
    )
    nc = _get_program()
    kw = {}
    if trace:
        kw["trace"] = True
        if trace_kwargs:
            kw.update(trace_kwargs)
    res = run_bass_kernel_spmd(nc, in_maps, list(range(NCORES)), **kw)
    out = _assemble(res.results, inputs["conv_b"])
    return out, res


def kernel(**inputs):
    out, _ = _run(inputs, trace=False)
    return out


# revision 12
# speedup vs baseline: 1.1460x; 1.0002x over previous
# Trainium2 Bass kernel for nn_LocalCrossAttentionModule.
#
# Math: softmax over a size-1 axis is identically 1, so q/k (and x_query,
# Wq, bq, Wk, bk) never affect the output. The module reduces to, per
# 5x5 patch p (576 of them = 4 batch x 12x12 grid, stride 36):
#   kvf_p  = flatten(x_kv patch)                  (3200,)
#   v_p    = Wv @ kvf_p + bv                      (1600,) viewed as (64, 5, 5)
#   z_p    = conv_w @ v_p[:, s] + conv_b          (128,) per pixel s in 5x5
# z_p is scattered into an otherwise-constant (conv_b) output map.
#
# Sharding: 4 feature-shards x 2 patch-halves across 8 cores. Each
# feature-shard owns 6 whole patch pixels (24 of 25); the 25th pixel is
# handled by folding the 1x1 conv into the weights host-side
# (W2 = conv_w @ Wv_p24), splitting its 128 conv-output rows 32 per
# feature-shard. Per-core device work is one fused [128, 25, 704] f16
# stream (weights 416 cols | kvf half 288 cols per k-tile).
#
# The device program is raw bacc (no TileContext): Tile's end-of-kernel
# semaphore butterfly costs ~10us of HW exec time, so semaphores are
# placed by hand (7 sems).

import numpy as np

B = 4
CKV = 128
HW_ = 432
E = 2
PP = 5           # patch side
STRIDE = 36
PI = 12          # patch grid side
NP = B * PI * PI      # 576 patches
KF = CKV * PP * PP    # 3200 kv features per patch
KT = KF // 128        # 25 contraction k-tiles
OUT = 64
O2 = 128
NCORES = 8
NF = 4           # feature shards
NPX = 6          # whole pixels per feature shard
WCOLS = NPX * OUT + 32   # 416 weight cols (384 v-rows + 32 folded z-rows)
NCH = NP // 2    # 288 patches per core (half)
WKC = WCOLS + NCH        # 704 (per k-tile: [kvf 288 | w 416])
CHUNKS = [1, 2, 4, 5, 5, 4, 2, 1, 1]  # k-tiles per input DMA (sum 25)
N_WARM = 13      # PE warm-up matmuls (cold ~240ns each ~ 3.1us)

_PROGRAM = {}


def _build_program():
    import concourse.mybir as mybir
    from concourse import bacc
    from concourse import bass as bassmod
    from contextlib import contextmanager

    @contextmanager
    def open_block(nc, name):
        # BassBlock without the exit all_engine_barrier: each engine flows
        # straight into the framework's end-of-kernel semaphore walk when
        # its own stream ends, overlapping the walk with other engines'
        # tails. Engines whose walk subset contains our sems (GpSimd:
        # 105-155, Vector: 156-206) end with a wait on store completion.
        assert nc.cur_block is None
        blk = bassmod.BassBlock(nc, name)
        blk.__enter__()
        nc.cur_block = blk
        try:
            yield blk
        finally:
            for engine, last_body in blk.last_body.items():
                with nc.body(last_body, parent=nc.cur_bb,
                             allow_existing_parent=True):
                    engine.br(blk.end_bb)
            nc.switch_bb(blk.end_bb)
            nc.cur_block = None

    f32 = mybir.dt.float32
    f16 = mybir.dt.float16
    add = mybir.AluOpType.add

    nc = bacc.Bacc()
    wk_d = nc.declare_dram_parameter("wk", [128, KT, WKC], f16, isOutput=False)
    cb_d = nc.declare_dram_parameter("cb", [128, 5], f32, isOutput=False)
    cw_d = nc.declare_dram_parameter("cw", [128, 128], f16, isOutput=False)
    z_d = nc.declare_dram_parameter("z", [128, NPX, NCH], f16, isOutput=True)
    z24_d = nc.declare_dram_parameter("z24", [32, NCH], f16, isOutput=True)

    # chunk index that must be complete before k-tile k is consumed
    need = []
    for ci, sz in enumerate(CHUNKS):
        need += [ci] * sz

    from contextlib import ExitStack

    with ExitStack() as stack:
        ec = stack.enter_context
        s_in = ec(nc.semaphore("s_in"))      # wk chunk completions (x16)
        s_c = ec(nc.semaphore("s_c"))        # const DMA completions (x16)
        s_pad = ec(nc.semaphore("s_pad"))    # junk-region memsets done
        s_pe = ec(nc.semaphore("s_pe"))      # mm1 accumulation done per m
        s_v = ec(nc.semaphore("s_v"))        # V extracted per m
        s_pe2 = ec(nc.semaphore("s_pe2"))    # mm2 done per pixel
        s_zd = ec(nc.semaphore("s_zd"))      # z pair written (DVE)
        s_z24 = ec(nc.semaphore("s_z24"))    # z24 written
        s_out = ec(nc.semaphore("s_out"))    # output DMA completions (x16)
        # flat layout; +128 pad cols so the last k-tile's widened m3
        # lhsT read (spills past the weights) stays in-bounds
        wk_t = ec(nc.sbuf_tensor("wk_t", [128, KT * WKC + 128], f16))
        cb_t = ec(nc.sbuf_tensor("cb_t", [128, 5], f32))
        cw_t = ec(nc.sbuf_tensor("cw_t", [128, 128], f16))
        warm_t = ec(nc.sbuf_tensor("warm_t", [128, NCH], f16))
        v_t = ec(nc.sbuf_tensor("v_t", [128, 3, NCH], f16))
        z_t = ec(nc.sbuf_tensor("z_t", [128, NPX, NCH], f16))
        z24_t = ec(nc.sbuf_tensor("z24_t", [32, NCH], f16))
        psv0 = ec(nc.psum_tensor("psv0", [128, NCH], f32))
        psv1 = ec(nc.psum_tensor("psv1", [128, NCH], f32))
        psv2 = ec(nc.psum_tensor("psv2", [128, NCH], f32))
        psv3 = ec(nc.psum_tensor("psv3", [128, NCH], f32))
        psz0 = ec(nc.psum_tensor("psz0", [128, NCH], f32))
        psz1 = ec(nc.psum_tensor("psz1", [128, NCH], f32))
        psz2 = ec(nc.psum_tensor("psz2", [128, NCH], f32))
        psz3 = ec(nc.psum_tensor("psz3", [128, NCH], f32))
        ps_v = [psv0[:], psv1[:], psv2[:], psv3[:]]
        # mm2 output banks: 4 fresh + reuse psv0/psv1 (their V is long
        # extracted by the time pixels 4/5 run, guarded by s_v)
        ps_z = [psz0[:], psz1[:], psz2[:], psz3[:], psv0[:], psv1[:]]

        with open_block(nc, "blk") as block:

            @block.sync
            def _(sync):
                lo = 0
                for sz in CHUNKS:
                    sync.dma_start(
                        wk_t[:, lo * WKC:(lo + sz) * WKC],
                        wk_d[:, lo:lo + sz, :],
                    ).then_inc(s_in, 16)
                    lo += sz
                sync.wait_ge(s_out, 16 * 4)

            @block.scalar
            def _(scalar):
                scalar.dma_start(cb_t[:], cb_d[:]).then_inc(s_c, 16)
                scalar.dma_start(cw_t[:], cw_d[:]).then_inc(s_c, 16)
                # stores: z24 first (ready earliest), then pixel pairs
                scalar.wait_ge(s_z24, 1)
                scalar.dma_start(z24_d[:], z24_t[:]).then_inc(s_out, 16)
                for j in range(3):
                    scalar.wait_ge(s_zd, 2 * j + 2)
                    scalar.dma_start(
                        z_d[:, 2 * j:2 * j + 2, :], z_t[:, 2 * j:2 * j + 2, :]
                    ).then_inc(s_out, 16)
                scalar.wait_ge(s_out, 16 * 4)

            @block.tensor
            def _(tensor):
                # warm-up: keeps PE_HAM busy through the first-chunk DMA
                # latency so real matmuls run at 2.4 GHz
                for _ in range(N_WARM):
                    tensor.matmul(
                        psz0[:], lhsT=warm_t[:, 0:128], rhs=warm_t[:],
                        start=True, stop=True,
                    )
                # mm1: V[f, n] accumulated over 25 k-tiles
                tensor.wait_ge(s_pad, 1)
                last_need = -1
                for k in range(KT):
                    if need[k] != last_need:
                        tensor.wait_ge(s_in, 16 * (need[k] + 1))
                        last_need = need[k]
                    base = k * WKC
                    for m in range(4):
                        # m3 is only 32 real cols (the conv-folded rows);
                        # widen to 128 by reading into the next k-tile's kvf
                        # region -- finite junk that lands in PSUM
                        # partitions 32-127, which nothing reads.
                        lo = base + NCH + m * 128
                        mm = tensor.matmul(
                            ps_v[m],
                            lhsT=wk_t[:, lo:lo + 128],
                            rhs=wk_t[:, base:base + NCH],
                            start=(k == 0),
                            stop=(k == KT - 1),
                        )
                        if k == KT - 1:
                            mm.then_inc(s_pe, 1)
                # mm2: z[o2, n] per pixel, contraction over 64 v-features
                tensor.wait_ge(s_c, 32)
                for p in range(NPX):
                    m, h = divmod(p, 2)
                    if h == 0:
                        tensor.wait_ge(s_v, m + 1)
                    tensor.matmul(
                        ps_z[p],
                        lhsT=cw_t[64 * h:64 * (h + 1), :],
                        rhs=v_t[64 * h:64 * (h + 1), m, :],
                        start=True, stop=True,
                    ).then_inc(s_pe2, 1)
                # the end-of-kernel walk clears runtime sems (2-53 on this
                # engine); hold it until all output DMA completed
                tensor.wait_ge(s_out, 16 * 4)

            @block.vector
            def _(vector):
                vector.memset(warm_t[:], 0.0)
                vector.memset(wk_t[:, KT * WKC:], 0.0).then_inc(s_pad, 1)
                vector.wait_ge(s_c, 16)
                for m in range(3):
                    vector.wait_ge(s_pe, m + 1)
                    vector.tensor_tensor(
                        out=v_t[:, m, :],
                        in0=ps_v[m],
                        in1=cb_t[:, m:m + 1].to_broadcast((128, NCH)),
                        op=add,
                    ).then_inc(s_v, 1)
                vector.wait_ge(s_pe, 4)
                vector.tensor_tensor(
                    out=z24_t[:],
                    in0=psv3[0:32, :],
                    in1=cb_t[0:32, 3:4].to_broadcast((32, NCH)),
                    op=add,
                ).then_inc(s_z24, 1)
                for p in range(NPX):
                    vector.wait_ge(s_pe2, p + 1)
                    vector.tensor_tensor(
                        out=z_t[:, p, :],
                        in0=ps_z[p],
                        in1=cb_t[:, 4:5].to_broadcast((128, NCH)),
                        op=add,
                    ).then_inc(s_zd, 1)
                vector.wait_ge(s_out, 16 * 4)

            @block.gpsimd
            def _(gpsimd):
                # hold GpSimd's end-walk (clears sems 105-155) until all
                # output DMAs completed; the walk itself restores sem state
                gpsimd.wait_ge(s_out, 16 * 4)

    nc.finalize()
    return nc


def _get_program():
    if "p" not in _PROGRAM:
        _PROGRAM["p"] = _build_program()
    return _PROGRAM["p"]


def _prep_in_maps(x_kv, Wv, bv, conv_w, conv_b):
    """Host-side shard/layout prep. Returns list of per-core input dicts."""
    x_kv = np.ascontiguousarray(np.asarray(x_kv, dtype=np.float32))
    Wv = np.asarray(Wv, dtype=np.float32)
    bv = np.asarray(bv, dtype=np.float32)
    conv_w = np.asarray(conv_w, dtype=np.float32)
    conv_b = np.asarray(conv_b, dtype=np.float32)

    # gather all 5x5 patches (padded coords: top-left of patch (pi,pj) is
    # original coords (pi*36-2, pj*36-2))
    pad = np.zeros((B, CKV, HW_ + 2 * E, HW_ + 2 * E), np.float32)
    pad[:, :, E:HW_ + E, E:HW_ + E] = x_kv
    r = (np.arange(PI)[:, None] * STRIDE + np.arange(PP)).ravel()  # (60,)
    g = pad[:, :, r[:, None], r[None, :]]                # (B, C, 60, 60)
    g = g.reshape(B, CKV, PI, PP, PI, PP)
    # feature j = c*25 + pr*5 + pc ; patch n = b*144 + pi*12 + pj
    kvf_t = g.transpose(1, 3, 5, 0, 2, 4).reshape(KF, NP)     # (3200, 576)
    kv_arr = kvf_t.reshape(KT, 128, NP).transpose(1, 0, 2)    # (128, 25, 576)
    kv_arr = np.ascontiguousarray(kv_arr).astype(np.float16)

    # conv folded into the 25th pixel's weights
    perm24 = np.array([o * PP * PP + 24 for o in range(OUT)], np.int64)
    W2 = conv_w @ Wv[perm24]                 # (128, 3200)
    b2 = conv_w @ bv[perm24] + conv_b        # (128,)

    # conv_w.T duplicated into both partition halves (mm2 lhsT must share
    # the rhs base partition)
    cw = np.ascontiguousarray(
        np.concatenate([conv_w.T, conv_w.T], axis=0)).astype(np.float16)

    in_maps = [None] * NCORES
    for f in range(NF):
        pixels = range(NPX * f, NPX * (f + 1))
        perm = np.array(
            [o * PP * PP + s for s in pixels for o in range(OUT)], np.int64
        )  # 384, layout j = s_local*64 + o
        A = np.concatenate([Wv[perm], W2[32 * f:32 * (f + 1)]], axis=0)  # (416, 3200)
        lhsT = np.ascontiguousarray(A.T)                     # (3200, 416)
        w_arr = lhsT.reshape(KT, 128, WCOLS).transpose(1, 0, 2)  # (128, 25, 416)
        w_arr = np.ascontiguousarray(w_arr).astype(np.float16)

        cb = np.zeros((128, 5), np.float32)
        cb[:, 0:3] = bv[perm].reshape(3, 128).T
        cb[0:32, 3] = b2[32 * f:32 * (f + 1)]
        cb[:, 4] = conv_b

        for p in range(2):
            wk = np.concatenate(
                [kv_arr[:, :, NCH * p:NCH * (p + 1)], w_arr], axis=2
            )  # (128, 25, 704) f16, [kvf | w] per k-tile
            in_maps[2 * f + p] = {
                "wk": np.ascontiguousarray(wk),
                "cb": cb,
                "cw": cw,
            }
    return in_maps


def _assemble(results, conv_b, out_dtype=np.float32):
    """Scatter per-core z outputs into the full (B, 128, 432, 432) map."""
    conv_b = np.asarray(conv_b, dtype=np.float32)
    y = np.empty((B, O2, HW_, HW_), np.float32)
    y[:] = conv_b.reshape(1, O2, 1, 1)
    base = np.arange(PI) * STRIDE
    for c in range(NCORES):
        f, p = divmod(c, 2)
        bs = slice(2 * p, 2 * p + 2)  # patch half p covers batches 2p, 2p+1
        z = np.asarray(results[c]["z"], np.float32)      # (128, 6, 288)
        for sl, s in enumerate(range(NPX * f, NPX * (f + 1))):
            pr, pc = divmod(s, PP)
            blk = z[:, sl, :].reshape(O2, 2, PI, PI).transpose(1, 0, 2, 3)
            y[bs, :, (base + pr)[:, None], (base + pc)[None, :]] = blk
        z24 = np.asarray(results[c]["z24"], np.float32)  # (32, 288)
        blk = z24.reshape(32, 2, PI, PI).transpose(1, 0, 2, 3)
        y[bs, 32 * f:32 * (f + 1),
          (base + PP - 1)[:, None], (base + PP - 1)[None, :]] = blk
    return y.astype(out_dtype, copy=False)


def _run(inputs, trace=False, trace_kwargs=None):
    from concourse.bass_utils import run_bass_kernel_spmd

    in_maps = _prep_in_maps(
        inputs["x_kv"], inputs["Wv"], inputs["bv"],
        inputs["conv_w"], inputs["conv_b"],
    )
    nc = _get_program()
    kw = {}
    if trace:
        kw["trace"] = True
        if trace_kwargs:
            kw.update(trace_kwargs)
    res = run_bass_kernel_spmd(nc, in_maps, list(range(NCORES)), **kw)
    out = _assemble(res.results, inputs["conv_b"])
    return out, res


def kernel(**inputs):
    out, _ = _run(inputs, trace=False)
    return out


# revision 13
# speedup vs baseline: 1.1687x; 1.0198x over previous
# Trainium2 Bass kernel for nn_LocalCrossAttentionModule.
#
# Math: softmax over a size-1 axis is identically 1, so q/k (and x_query,
# Wq, bq, Wk, bk) never affect the output. The module reduces to, per
# 5x5 patch p (576 of them = 4 batch x 12x12 grid, stride 36):
#   kvf_p  = flatten(x_kv patch)                  (3200,)
#   v_p    = Wv @ kvf_p + bv                      (1600,) viewed as (64, 5, 5)
#   z_p    = conv_w @ v_p[:, s] + conv_b          (128,) per pixel s in 5x5
# z_p is scattered into an otherwise-constant (conv_b) output map.
#
# Sharding: 4 feature-shards x 2 patch-halves across 8 cores. Each
# feature-shard owns 6 whole patch pixels (24 of 25); the 25th pixel is
# handled by folding the 1x1 conv into the weights host-side
# (W2 = conv_w @ Wv_p24), splitting its 128 conv-output rows 32 per
# feature-shard. Per-core device work is one fused [128, 25, 704] f16
# stream (weights 416 cols | kvf half 288 cols per k-tile).
#
# The device program is raw bacc (no TileContext): Tile's end-of-kernel
# semaphore butterfly costs ~10us of HW exec time, so semaphores are
# placed by hand (7 sems).

import numpy as np

B = 4
CKV = 128
HW_ = 432
E = 2
PP = 5           # patch side
STRIDE = 36
PI = 12          # patch grid side
NP = B * PI * PI      # 576 patches
KF = CKV * PP * PP    # 3200 kv features per patch
KT = KF // 128        # 25 contraction k-tiles
OUT = 64
O2 = 128
NCORES = 8
NF = 4           # feature shards
NPX = 6          # whole pixels per feature shard
WCOLS = NPX * OUT + 32   # 416 weight cols (384 v-rows + 32 folded z-rows)
NCH = NP // 2    # 288 patches per core (half)
WKC = WCOLS + NCH        # 704 (per k-tile: [kvf 288 | w 416])
CHUNKS = [2, 5, 6, 6, 5, 1]  # k-tiles per input DMA (sum 25)
N_WARM = 13      # PE warm-up matmuls (cold ~240ns each ~ 3.1us)

_PROGRAM = {}


def _build_program():
    import concourse.mybir as mybir
    from concourse import bacc
    from concourse import bass as bassmod
    from contextlib import contextmanager

    @contextmanager
    def open_block(nc, name):
        # BassBlock without the exit all_engine_barrier: each engine flows
        # straight into the framework's end-of-kernel semaphore walk when
        # its own stream ends, overlapping the walk with other engines'
        # tails. Engines whose walk subset contains our sems (GpSimd:
        # 105-155, Vector: 156-206) end with a wait on store completion.
        assert nc.cur_block is None
        blk = bassmod.BassBlock(nc, name)
        blk.__enter__()
        nc.cur_block = blk
        try:
            yield blk
        finally:
            for engine, last_body in blk.last_body.items():
                with nc.body(last_body, parent=nc.cur_bb,
                             allow_existing_parent=True):
                    engine.br(blk.end_bb)
            nc.switch_bb(blk.end_bb)
            nc.cur_block = None

    f32 = mybir.dt.float32
    f16 = mybir.dt.float16
    add = mybir.AluOpType.add

    nc = bacc.Bacc()
    wk_d = nc.declare_dram_parameter("wk", [128, KT, WKC], f16, isOutput=False)
    cb_d = nc.declare_dram_parameter("cb", [128, 5], f32, isOutput=False)
    cw_d = nc.declare_dram_parameter("cw", [128, 128], f16, isOutput=False)
    z_d = nc.declare_dram_parameter("z", [128, NPX, NCH], f16, isOutput=True)
    z24_d = nc.declare_dram_parameter("z24", [32, NCH], f16, isOutput=True)

    # chunk index that must be complete before k-tile k is consumed
    need = []
    for ci, sz in enumerate(CHUNKS):
        need += [ci] * sz

    from contextlib import ExitStack

    with ExitStack() as stack:
        ec = stack.enter_context
        s_in = ec(nc.semaphore("s_in"))      # wk chunk completions (x16)
        s_c = ec(nc.semaphore("s_c"))        # const DMA completions (x16)
        s_pad = ec(nc.semaphore("s_pad"))    # junk-region memsets done
        s_pe = ec(nc.semaphore("s_pe"))      # mm1 accumulation done per m
        s_v = ec(nc.semaphore("s_v"))        # V extracted per m
        s_pe2 = ec(nc.semaphore("s_pe2"))    # mm2 done per pixel
        s_zd = ec(nc.semaphore("s_zd"))      # z pair written (DVE)
        s_z24 = ec(nc.semaphore("s_z24"))    # z24 written
        s_out = ec(nc.semaphore("s_out"))    # output DMA completions (x16)
        # flat layout; +128 pad cols so the last k-tile's widened m3
        # lhsT read (spills past the weights) stays in-bounds
        wk_t = ec(nc.sbuf_tensor("wk_t", [128, KT * WKC + 128], f16))
        cb_t = ec(nc.sbuf_tensor("cb_t", [128, 5], f32))
        cw_t = ec(nc.sbuf_tensor("cw_t", [128, 128], f16))
        warm_t = ec(nc.sbuf_tensor("warm_t", [128, NCH], f16))
        v_t = ec(nc.sbuf_tensor("v_t", [128, 3, NCH], f16))
        z_t = ec(nc.sbuf_tensor("z_t", [128, NPX, NCH], f16))
        z24_t = ec(nc.sbuf_tensor("z24_t", [32, NCH], f16))
        psv0 = ec(nc.psum_tensor("psv0", [128, NCH], f32))
        psv1 = ec(nc.psum_tensor("psv1", [128, NCH], f32))
        psv2 = ec(nc.psum_tensor("psv2", [128, NCH], f32))
        psv3 = ec(nc.psum_tensor("psv3", [128, NCH], f32))
        psz0 = ec(nc.psum_tensor("psz0", [128, NCH], f32))
        psz1 = ec(nc.psum_tensor("psz1", [128, NCH], f32))
        psz2 = ec(nc.psum_tensor("psz2", [128, NCH], f32))
        psz3 = ec(nc.psum_tensor("psz3", [128, NCH], f32))
        ps_v = [psv0[:], psv1[:], psv2[:], psv3[:]]
        # mm2 output banks: 4 fresh + reuse psv0/psv1 (their V is long
        # extracted by the time pixels 4/5 run, guarded by s_v)
        ps_z = [psz0[:], psz1[:], psz2[:], psz3[:], psv0[:], psv1[:]]

        with open_block(nc, "blk") as block:

            @block.sync
            def _(sync):
                lo = 0
                for sz in CHUNKS:
                    sync.dma_start(
                        wk_t[:, lo * WKC:(lo + sz) * WKC],
                        wk_d[:, lo:lo + sz, :],
                    ).then_inc(s_in, 16)
                    lo += sz
                sync.wait_ge(s_out, 16 * 4)

            @block.scalar
            def _(scalar):
                scalar.dma_start(cb_t[:], cb_d[:]).then_inc(s_c, 16)
                scalar.dma_start(cw_t[:], cw_d[:]).then_inc(s_c, 16)
                # stores: z24 first (ready earliest), then pixel pairs
                scalar.wait_ge(s_z24, 1)
                scalar.dma_start(z24_d[:], z24_t[:]).then_inc(s_out, 16)
                for j in range(3):
                    scalar.wait_ge(s_zd, 2 * j + 2)
                    scalar.dma_start(
                        z_d[:, 2 * j:2 * j + 2, :], z_t[:, 2 * j:2 * j + 2, :]
                    ).then_inc(s_out, 16)
                scalar.wait_ge(s_out, 16 * 4)

            @block.tensor
            def _(tensor):
                # warm-up: keeps PE_HAM busy through the first-chunk DMA
                # latency so real matmuls run at 2.4 GHz
                for _ in range(N_WARM):
                    tensor.matmul(
                        psz0[:], lhsT=warm_t[:, 0:128], rhs=warm_t[:],
                        start=True, stop=True,
                    )
                # mm1: V[f, n] accumulated over 25 k-tiles
                tensor.wait_ge(s_pad, 1)
                last_need = -1
                for k in range(KT):
                    if need[k] != last_need:
                        tensor.wait_ge(s_in, 16 * (need[k] + 1))
                        last_need = need[k]
                    base = k * WKC
                    for m in range(4):
                        # m3 is only 32 real cols (the conv-folded rows);
                        # widen to 128 by reading into the next k-tile's kvf
                        # region -- finite junk that lands in PSUM
                        # partitions 32-127, which nothing reads.
                        lo = base + NCH + m * 128
                        mm = tensor.matmul(
                            ps_v[m],
                            lhsT=wk_t[:, lo:lo + 128],
                            rhs=wk_t[:, base:base + NCH],
                            start=(k == 0),
                            stop=(k == KT - 1),
                        )
                        if k == KT - 1:
                            mm.then_inc(s_pe, 1)
                # mm2: z[o2, n] per pixel, contraction over 64 v-features
                tensor.wait_ge(s_c, 32)
                for p in range(NPX):
                    m, h = divmod(p, 2)
                    if h == 0:
                        tensor.wait_ge(s_v, m + 1)
                    tensor.matmul(
                        ps_z[p],
                        lhsT=cw_t[64 * h:64 * (h + 1), :],
                        rhs=v_t[64 * h:64 * (h + 1), m, :],
                        start=True, stop=True,
                    ).then_inc(s_pe2, 1)
                # the end-of-kernel walk clears runtime sems (2-53 on this
                # engine); hold it until all output DMA completed
                tensor.wait_ge(s_out, 16 * 4)

            @block.vector
            def _(vector):
                vector.memset(warm_t[:], 0.0)
                vector.memset(wk_t[:, KT * WKC:], 0.0).then_inc(s_pad, 1)
                vector.wait_ge(s_c, 16)
                for m in range(3):
                    vector.wait_ge(s_pe, m + 1)
                    vector.tensor_tensor(
                        out=v_t[:, m, :],
                        in0=ps_v[m],
                        in1=cb_t[:, m:m + 1].to_broadcast((128, NCH)),
                        op=add,
                    ).then_inc(s_v, 1)
                vector.wait_ge(s_pe, 4)
                vector.tensor_tensor(
                    out=z24_t[:],
                    in0=psv3[0:32, :],
                    in1=cb_t[0:32, 3:4].to_broadcast((32, NCH)),
                    op=add,
                ).then_inc(s_z24, 1)
                for p in range(NPX):
                    vector.wait_ge(s_pe2, p + 1)
                    vector.tensor_tensor(
                        out=z_t[:, p, :],
                        in0=ps_z[p],
                        in1=cb_t[:, 4:5].to_broadcast((128, NCH)),
                        op=add,
                    ).then_inc(s_zd, 1)
                vector.wait_ge(s_out, 16 * 4)

            @block.gpsimd
            def _(gpsimd):
                # hold GpSimd's end-walk (clears sems 105-155) until all
                # output DMAs completed; the walk itself restores sem state
                gpsimd.wait_ge(s_out, 16 * 4)

    nc.finalize()
    return nc


def _get_program():
    if "p" not in _PROGRAM:
        _PROGRAM["p"] = _build_program()
    return _PROGRAM["p"]


def _prep_in_maps(x_kv, Wv, bv, conv_w, conv_b):
    """Host-side shard/layout prep. Returns list of per-core input dicts."""
    x_kv = np.ascontiguousarray(np.asarray(x_kv, dtype=np.float32))
    Wv = np.asarray(Wv, dtype=np.float32)
    bv = np.asarray(bv, dtype=np.float32)
    conv_w = np.asarray(conv_w, dtype=np.float32)
    conv_b = np.asarray(conv_b, dtype=np.float32)

    # gather all 5x5 patches (padded coords: top-left of patch (pi,pj) is
    # original coords (pi*36-2, pj*36-2))
    pad = np.zeros((B, CKV, HW_ + 2 * E, HW_ + 2 * E), np.float32)
    pad[:, :, E:HW_ + E, E:HW_ + E] = x_kv
    r = (np.arange(PI)[:, None] * STRIDE + np.arange(PP)).ravel()  # (60,)
    g = pad[:, :, r[:, None], r[None, :]]                # (B, C, 60, 60)
    g = g.reshape(B, CKV, PI, PP, PI, PP)
    # feature j = c*25 + pr*5 + pc ; patch n = b*144 + pi*12 + pj
    kvf_t = g.transpose(1, 3, 5, 0, 2, 4).reshape(KF, NP)     # (3200, 576)
    kv_arr = kvf_t.reshape(KT, 128, NP).transpose(1, 0, 2)    # (128, 25, 576)
    kv_arr = np.ascontiguousarray(kv_arr).astype(np.float16)

    # conv folded into the 25th pixel's weights
    perm24 = np.array([o * PP * PP + 24 for o in range(OUT)], np.int64)
    W2 = conv_w @ Wv[perm24]                 # (128, 3200)
    b2 = conv_w @ bv[perm24] + conv_b        # (128,)

    # conv_w.T duplicated into both partition halves (mm2 lhsT must share
    # the rhs base partition)
    cw = np.ascontiguousarray(
        np.concatenate([conv_w.T, conv_w.T], axis=0)).astype(np.float16)

    in_maps = [None] * NCORES
    for f in range(NF):
        pixels = range(NPX * f, NPX * (f + 1))
        perm = np.array(
            [o * PP * PP + s for s in pixels for o in range(OUT)], np.int64
        )  # 384, layout j = s_local*64 + o
        A = np.concatenate([Wv[perm], W2[32 * f:32 * (f + 1)]], axis=0)  # (416, 3200)
        lhsT = np.ascontiguousarray(A.T)                     # (3200, 416)
        w_arr = lhsT.reshape(KT, 128, WCOLS).transpose(1, 0, 2)  # (128, 25, 416)
        w_arr = np.ascontiguousarray(w_arr).astype(np.float16)

        cb = np.zeros((128, 5), np.float32)
        cb[:, 0:3] = bv[perm].reshape(3, 128).T
        cb[0:32, 3] = b2[32 * f:32 * (f + 1)]
        cb[:, 4] = conv_b

        for p in range(2):
            wk = np.concatenate(
                [kv_arr[:, :, NCH * p:NCH * (p + 1)], w_arr], axis=2
            )  # (128, 25, 704) f16, [kvf | w] per k-tile
            in_maps[2 * f + p] = {
                "wk": np.ascontiguousarray(wk),
                "cb": cb,
                "cw": cw,
            }
    return in_maps


def _assemble(results, conv_b, out_dtype=np.float32):
    """Scatter per-core z outputs into the full (B, 128, 432, 432) map."""
    conv_b = np.asarray(conv_b, dtype=np.float32)
    y = np.empty((B, O2, HW_, HW_), np.float32)
    y[:] = conv_b.reshape(1, O2, 1, 1)
    base = np.arange(PI) * STRIDE
    for c in range(NCORES):
        f, p = divmod(c, 2)
        bs = slice(2 * p, 2 * p + 2)  # patch half p covers batches 2p, 2p+1
        z = np.asarray(results[c]["z"], np.float32)      # (128, 6, 288)
        for sl, s in enumerate(range(NPX * f, NPX * (f + 1))):
            pr, pc = divmod(s, PP)
            blk = z[:, sl, :].reshape(O2, 2, PI, PI).transpose(1, 0, 2, 3)
            y[bs, :, (base + pr)[:, None], (base + pc)[None, :]] = blk
        z24 = np.asarray(results[c]["z24"], np.float32)  # (32, 288)
        blk = z24.reshape(32, 2, PI, PI).transpose(1, 0, 2, 3)
        y[bs, 32 * f:32 * (f + 1),
          (base + PP - 1)[:, None], (base + PP - 1)[None, :]] = blk
    return y.astype(out_dtype, copy=False)


def _run(inputs, trace=False, trace_kwargs=None):
    from concourse.bass_utils import run_bass_kernel_spmd

    in_maps = _prep_in_maps(
        inputs["x_kv"], inputs["Wv"], inputs["bv"],
        inputs["conv_w"], inputs["conv_b"],
    )
    nc = _get_program()
    kw = {}
    if trace:
        kw["trace"] = True
        if trace_kwargs:
            kw.update(trace_kwargs)
    res = run_bass_kernel_spmd(nc, in_maps, list(range(NCORES)), **kw)
    out = _assemble(res.results, inputs["conv_b"])
    return out, res


def kernel(**inputs):
    out, _ = _run(inputs, trace=False)
    return out
